# revision 17
# baseline (speedup 1.0000x reference)
"""Trainium2 Bass kernel for nn_BruteForceUpdater (fp16 GEMM + stream scan).

Reference computation:
    xs = x[:, 0, :]                       # [256, 128]
    U  = (xs @ W1.T) @ W2.T               # [256, 8256]
    fw_{i+1} = sigmoid(10*(fw_i + U_i - 0.5))   (serial over batch)
    pred_i = fw2_i @ relu(fw1_i @ x_i)    # fw1 = fw[:8192].reshape(64,128)

Distribution over 8 NeuronCores (no collectives; host sums partials):
  * Core c owns fast-w1 row-tiles h in [8c, 8c+8) (1024 rows of W2); every
    core replicates the trailing 64 rows (the fast-w2 part) -> each core
    processes 1088 W2 rows = 9 output tiles (NT) of U.T.
  * Streamed tensor wq: k-blocked weights, fp16, 3 k-tiles per DMA
    ([128, 3*1216]): cols 0:1088 of each k-tile = the W2 slice
    pre-transposed, cols 1088:1216 = the k-block-transposed W1 chunk
    (T1 = W1 @ xs.T is produced on device 2 k-tiles ahead of the GEMM).
  * GEMM: fp16 operands (error ~5e-4 << the 2e-2 gate), fp32 psum
    accumulation over 129 k-tiles into U.T [9 tiles x 256].
  * Scan: the gain-10 sigmoid recurrence contracts hard, so the 256-step
    chain splits into 16 chunks; chunks >=2 rerun 24 warmup steps from
    state 0.5 (numpy-validated: reproduces the exact scan to ~5e-4).
    Chunks are merged into 4 independent streams (uniform col stride 48)
    so one DVE add + one ACT sigmoid advances 4-5 chunks at once; streams
    interleave round-robin, hiding the cross-engine dependency latency.
  * Prediction: prods fw1*x per m-tile (split Pool/DVE), 8 one-hot
    "sel" matmuls place each tile's column sums at psum partition 8c+m,
    one fused max(H,0)*fw2 op, one ones-matmul -> [1,256] partials.

Hand-rolled semaphores (one wait + one inc per instruction, walrus
limit); in-order engine queues carry the rest of the ordering.
"""
import os
import sys

sys.path.insert(0, "/opt/trn_rl_repo")

import numpy as np
from contextlib import ExitStack

import concourse.bass as bass
import concourse.tile as tile
from concourse import mybir
from concourse.bass_utils import run_bass_kernel_spmd

F32 = mybir.dt.float32
F16 = mybir.dt.float16
AF = mybir.ActivationFunctionType
ALU = mybir.AluOpType

IN = 128
HID = 64
NFW = IN * HID + HID          # 8256
B = 256
K2 = 2 * NFW                  # 16512
KT = K2 // 128                # 129 contraction tiles
NCORES = 8
MT_OWN = 8                    # full 128-row W2 tiles owned per core
NT = MT_OWN + 1               # + shared 64-row tile
MSL = MT_OWN * 128 + HID      # 1088 W2 rows handled per core
WC = MSL + 128                # k-tile width (W2 cols + W1 chunk)
GAIN, SHIFT = 10.0, 0.5

T3 = 3                        # k-tiles per streamed DMA
QT = KT // T3                 # 43 triple-DMAs
NQ = 8                        # triple ring slots
LOOKQ = 7                     # triples issued ahead
NT1 = 4                       # t1 ring slots

# scan streams: chunk 0 = cols 0:32 (exact, from fw0); chunks j=2..15 =
# cols 16j:16j+16 with 24 warmup steps from 0.5.  Streams group chunks
# at uniform stride 48 so one instruction advances a whole stream.
HWRM = 24
LC0 = 32                      # chunk-0 columns (rounds 0..31)
STREAMS = [[2, 5, 8, 11, 14], [3, 6, 9, 12, 15], [4, 7, 10, 13]]
ROUNDS = HWRM + 16            # 40

_NC_CACHE = None


def _build_bass():
    nc = bass.Bass("TRN2", target_bir_lowering=False, debug=False)

    wq_d = nc.dram_tensor("wq", [QT * 128, T3 * WC], F16, kind="ExternalInput")
    cst_d = nc.dram_tensor("cst", [128, B + NT], F32, kind="ExternalInput")
    sel_d = nc.dram_tensor("sel", [128, MT_OWN * HID + 1], F16,
                           kind="ExternalInput")
    pred_d = nc.dram_tensor("pred", [1, B], F32, kind="ExternalOutput")
    dbg = bool(int(os.environ.get("KDBG", "0")))
    if dbg:
        u_dbg_d = nc.dram_tensor("u_dbg", [128, NT * B], F32,
                                 kind="ExternalOutput")
        fw_dbg_d = nc.dram_tensor("fw_dbg", [128, NT * B], F32,
                                  kind="ExternalOutput")
        q_dbg_d = nc.dram_tensor("q_dbg", [HID, B], F16,
                                 kind="ExternalOutput")

    with tile.TileContext(nc) as tc:
        with ExitStack() as ctx:
            const_pool = ctx.enter_context(tc.tile_pool(name="const", bufs=1))
            stream_pool = ctx.enter_context(tc.tile_pool(name="wcs", bufs=1))
            big_pool = ctx.enter_context(tc.tile_pool(name="big", bufs=1))

            cst = const_pool.tile([128, B + NT], F32)
            xst = cst[:, 0:B]
            fw0_t = cst[:, B:B + NT]
            sel = const_pool.tile([128, MT_OWN * HID + 1], F16)
            ones16 = sel[:, MT_OWN * HID:MT_OWN * HID + 1]
            xst_h = const_pool.tile([128, B], F16)
            bias_t = const_pool.tile([128, 1], F32)
            half_t = const_pool.tile([128, NT], F32)
            zf16 = const_pool.tile([128, 128], F16)

            wbuf = stream_pool.tile([128, NQ * T3 * WC], F16)  # stream ring
            t1r = big_pool.tile([128, NT1 * B], F16)           # T1.T ring
            u_sb = big_pool.tile([128, NT * B], F32)           # U.T slice
            fw_sb = big_pool.tile([128, NT * B], F32)          # fw history
            # per-stream add/sigmoid ping-pong + warmup state slots
            t_big = big_pool.tile([128, 2 * NT * (1 + 5 + 5 + 4)], F32)
            wsc = big_pool.tile([128, 2 * NT * (5 + 5 + 4)], F32)
            prod_h = big_pool.tile([128, MT_OWN * B], F16)     # fw1*x
            q_sb = big_pool.tile([HID, B], F16)                # relu(H)*fw2
            pred_sb = big_pool.tile([1, B], F32)

            u_r = u_sb[:].rearrange("p (m i) -> p m i", m=NT)
            fw_r = fw_sb[:].rearrange("p (m i) -> p m i", m=NT)

            def qslot(q):
                s = q % NQ
                return wbuf[:, s * T3 * WC:(s + 1) * T3 * WC]

            def wslot(k):
                s = (k // T3) % NQ
                off = (s * T3 + k % T3) * WC
                return wbuf[:, off:off + WC]

            def t1slot(j):
                s = j % NT1
                return t1r[:, s * B:(s + 1) * B]

            # sub-bank psum packing for the 9 accumulating U.T tiles
            tile_bytes = B * 4
            bankof = [m * tile_bytes // 2048 for m in range(NT)]
            m_first = [m == 0 or bankof[m] != bankof[m - 1] for m in range(NT)]
            m_last = [m == NT - 1 or bankof[m] != bankof[m + 1] for m in range(NT)]

            csem = nc.alloc_semaphore("csem")
            dsem = [nc.alloc_semaphore(f"dsem{s}") for s in range(NQ)]
            tmm_sem = nc.alloc_semaphore("tmm")
            cp_sem = nc.alloc_semaphore("cp")
            pe_sem = nc.alloc_semaphore("pe")
            sv = nc.alloc_semaphore("sv")     # DVE progress
            sa = nc.alloc_semaphore("sa")     # ACT progress
            pl = nc.alloc_semaphore("pl")     # Pool progress
            pp = nc.alloc_semaphore("pp")     # PE pred progress
            dsm = nc.alloc_semaphore("dsm")

            with tc.tile_pool(name="pt1", bufs=1, space="PSUM") as pt_pool, \
                 tc.tile_pool(name="pu", bufs=1, space="PSUM") as pu_pool:
                pt = pt_pool.tile([128, 1024], F32)
                psum_u = pu_pool.tile([128, NT * B], F32)
                pu_r = psum_u[:].rearrange("p (m i) -> p m i", m=NT)

                with tc.tile_critical():
                    svc = [0]                 # sv value tracker

                    def dve_inc(inst):
                        inst.then_inc(sv, 1)
                        svc[0] += 1
                        return svc[0]

                    # constants: cst rides the fast Sync hardware queue (a
                    # software-queue DMA costs ~5us completion latency and
                    # gates the whole startup); sel stays on the Pool queue
                    # (only needed post-scan).
                    nc.sync.dma_start(cst[:], cst_d[:, :]).then_inc(csem, 16)
                    nc.gpsimd.dma_start(sel[:], sel_d[:, :]).then_inc(csem, 16)
                    nc.vector.memset(bias_t[:], -GAIN * SHIFT)
                    dve_inc(nc.vector.memset(half_t[:], 0.5))
                    dve_inc(nc.vector.memset(zf16[:], 0.0))
                    cxr = nc.vector.tensor_copy(xst_h[:], xst)
                    cxr._wait_ge(csem, 16)
                    v_xr = dve_inc(cxr)

                    def dma_q(q):
                        d = nc.sync.dma_start(
                            qslot(q), wq_d[q * 128:(q + 1) * 128, :])
                        if q >= NQ:
                            d._wait_ge(pe_sem, T3 * (q - NQ) + T3)
                        d.then_inc(dsem[q % NQ], 16)

                    def t1_mm(j):
                        mm = nc.tensor.matmul(
                            pt[:, (j % 2) * 512:(j % 2) * 512 + B],
                            wslot(j)[:, MSL:WC], xst_h[:],
                            start=True, stop=True)
                        q = j // T3
                        mm._wait_ge(dsem[q % NQ], 16 * (q // NQ + 1))
                        mm.then_inc(tmm_sem, 1)

                    def t1_copy(j):
                        cp = nc.vector.tensor_copy(
                            t1slot(j), pt[:, (j % 2) * 512:(j % 2) * 512 + B])
                        cp._wait_ge(tmm_sem, j + 1)
                        cp.then_inc(cp_sem, 1)

                    # PE warmup (pulls the const prep tick into PE's clock)
                    zmm = nc.tensor.matmul(pt[:, 0:B], zf16[:], xst_h[:],
                                           start=True, stop=True)
                    zmm._wait_ge(sv, v_xr)

                    for q in range(LOOKQ):
                        dma_q(q)
                    for j in range(2):
                        t1_mm(j)
                        t1_copy(j)

                    for k in range(KT):
                        if k % T3 == 0 and k // T3 + LOOKQ < QT:
                            dma_q(k // T3 + LOOKQ)
                        for m in range(NT):
                            mm = nc.tensor.matmul(
                                psum_u[:, m * B:(m + 1) * B],
                                wslot(k)[:, m * 128:(m + 1) * 128],
                                t1slot(k),
                                start=(k == 0 and m_first[m]),
                                stop=(k == KT - 1 and m_last[m]),
                            )
                            if m == 0:
                                mm._wait_ge(cp_sem, k + 1)
                            if m == NT - 1:
                                mm.then_inc(pe_sem, 1)
                        if k + 2 < KT:
                            t1_mm(k + 2)
                            t1_copy(k + 2)

                    # gate: sel-DMA arrival folded into the DVE stream here
                    # (long after issue; everything downstream inherits it)
                    g = nc.vector.memset(t_big[0:1, 0:1], 0.0)
                    g._wait_ge(csem, 32)
                    dve_inc(g)

                    # psum U -> sbuf, split DVE/ACT to halve the stall
                    ucp_d = nc.vector.tensor_copy(
                        u_sb[:, 0:NT * B // 2], psum_u[:, 0:NT * B // 2])
                    ucp_d._wait_ge(pe_sem, KT)
                    dve_inc(ucp_d)
                    ucp_a = nc.scalar.activation(
                        u_sb[:, NT * B // 2:], psum_u[:, NT * B // 2:],
                        AF.Copy)
                    ucp_a._wait_ge(pe_sem, KT)
                    ucp_a.then_inc(sa, 1)

                    # ---- stream scan ----
                    # stream 0: chunk 0 [128, NT]; streams 1-3: merged
                    # chunks [128, NT, n] at col stride 48.
                    sdesc = []      # (i0_base, n, t_off, w_off)
                    t_off = 0
                    w_off = 0
                    sdesc.append((0, 1, t_off, None))
                    t_off += 2 * NT
                    for s in STREAMS:
                        sdesc.append((16 * s[0] - HWRM, len(s), t_off, w_off))
                        t_off += 2 * NT * len(s)
                        w_off += 2 * NT * len(s)

                    def tview(si, r):
                        i0, n, to, _ = sdesc[si]
                        off = to + (r % 2) * NT * n
                        return t_big[:, off:off + NT * n].rearrange(
                            "p (m t) -> p m t", t=n)

                    def wview(si, r):
                        i0, n, _, wo = sdesc[si]
                        off = wo + (r % 2) * NT * n
                        return wsc[:, off:off + NT * n].rearrange(
                            "p (m t) -> p m t", t=n)

                    def fview(ap, si, r):
                        i0, n, _, _ = sdesc[si]
                        i = i0 + r
                        return ap[:, :, i:i + 48 * (n - 1) + 1:48]

                    v_add = {}
                    a_idx = {}
                    a_cnt = [1]               # ucp_a took sa slot 1

                    def emit_add(si, r):
                        i0, n, _, _ = sdesc[si]
                        if r == 0:
                            prev = (fw0_t[:, :, None] if si == 0
                                    else half_t[:, :, None]).broadcast_to(
                                        (128, NT, n))
                        elif si > 0 and r - 1 < HWRM:
                            prev = wview(si, r - 1)
                        else:
                            prev = fview(fw_r, si, r - 1)
                        add = nc.vector.tensor_add(
                            tview(si, r), prev, fview(u_r, si, r))
                        if r > 0:
                            add._wait_ge(sa, a_idx[(si, r - 1)])
                        else:
                            # round-0 adds read the ACT-copied half of U
                            add._wait_ge(sa, 1)
                        v_add[(si, r)] = dve_inc(add)

                    def emit_act(si, r):
                        out = (wview(si, r) if si > 0 and r < HWRM
                               else fview(fw_r, si, r))
                        act = nc.scalar.activation(
                            out, tview(si, r), AF.Sigmoid,
                            bias=bias_t[:], scale=GAIN)
                        act._wait_ge(sv, v_add[(si, r)])
                        act.then_inc(sa, 1)
                        a_cnt[0] += 1
                        a_idx[(si, r)] = a_cnt[0]

                    for r in range(ROUNDS):
                        for si in range(4):
                            if si == 0 and r >= LC0:
                                continue
                            emit_add(si, r)
                            emit_act(si, r)
                    a_total = a_cnt[0]

                    # keep the PE clock ramped through the scan so the
                    # prediction matmuls run at full pstate: one dummy
                    # matmul every 3 rounds, gated on scan progress
                    for r in range(2, ROUNDS, 3):
                        si = 1 if r >= LC0 else 0
                        wm = nc.tensor.matmul(
                            pt[:, 512:512 + B], zf16[:], xst_h[:],
                            start=True, stop=True)
                        wm._wait_ge(sa, a_idx[(si, r)])

                    # ---- prediction partials (all post-scan) ----
                    # prods split across DVE (odd m) and Pool (even m)
                    v_prod = {}
                    for m in (1, 3, 5, 7):
                        pr = nc.vector.tensor_mul(
                            prod_h[:, m * B:(m + 1) * B], fw_r[:, m, :],
                            xst_h[:])
                        if m == 1:
                            pr._wait_ge(sa, a_total)
                        v_prod[m] = dve_inc(pr)
                    for i, m in enumerate((0, 2, 4, 6)):
                        pr = nc.gpsimd.tensor_mul(
                            prod_h[:, m * B:(m + 1) * B], fw_r[:, m, :],
                            xst_h[:])
                        if m == 0:
                            pr._wait_ge(sa, a_total)
                        pr.then_inc(pl, 1)

                    # H[m,:] accumulated at psum partition 8*core+m via
                    # one-hot sel matmuls (all write the same [64,256] tile)
                    order = [1, 3, 5, 7, 0, 2, 4, 6]
                    for idx, m in enumerate(order):
                        hm = nc.tensor.matmul(
                            pt[0:HID, 0:B], sel[:, m * HID:(m + 1) * HID],
                            prod_h[:, m * B:(m + 1) * B],
                            start=(idx == 0), stop=(idx == len(order) - 1))
                        if m % 2 == 1:
                            hm._wait_ge(sv, v_prod[m])
                        else:
                            hm._wait_ge(pl, m // 2 + 1)
                        if idx == len(order) - 1:
                            hm.then_inc(pp, 1)

                    # q = max(H,0) * fw2  (fused, psum+sbuf -> sbuf fp16)
                    stt = nc.vector.scalar_tensor_tensor(
                        q_sb[:, :], pt[0:HID, 0:B], 0.0,
                        fw_r[0:HID, NT - 1, :], op0=ALU.max, op1=ALU.mult)
                    stt._wait_ge(pp, 1)
                    v_stt = dve_inc(stt)

                    pmm = nc.tensor.matmul(
                        psum_u[0:1, 0:B], ones16[0:HID, :], q_sb[:, :],
                        start=True, stop=True)
                    pmm._wait_ge(sv, v_stt)
                    pmm.then_inc(pp, 1)

                    cpd = nc.vector.tensor_copy(pred_sb[:], psum_u[0:1, 0:B])
                    cpd._wait_ge(pp, 2)
                    v_out = dve_inc(cpd)

                    dout = nc.sync.dma_start(pred_d[:, :], pred_sb[:])
                    dout._wait_ge(sv, v_out)
                    dout.then_inc(dsm, 16)
                    if dbg:
                        du = nc.sync.dma_start(u_dbg_d[:, :], u_sb[:])
                        du._wait_ge(sv, v_out)
                        du.then_inc(dsm, 16)
                        df = nc.sync.dma_start(fw_dbg_d[:, :], fw_sb[:])
                        df._wait_ge(sv, v_out)
                        df.then_inc(dsm, 16)
                        dq = nc.sync.dma_start(q_dbg_d[:, :], q_sb[:])
                        dq._wait_ge(sv, v_out)
                        dq.then_inc(dsm, 16)

    _dedupe_waits(nc)
    return nc


def _dedupe_waits(nc):
    """Collapse duplicate semaphore waits the framework occasionally emits
    (walrus allows very few sync commands per instruction)."""
    for fnn in nc.m.functions:
        for blk in fnn.blocks:
            for inst in blk.instructions:
                si = inst.sync_info
                if si is None or not si.on_wait or len(si.on_wait) < 2:
                    continue
                best = {}
                order = []
                for w in si.on_wait:
                    if w.wait_reg is not None or w.wait_mode != "sem-ge-imm":
                        key = ("raw", id(w))
                    else:
                        key = (w.sync_type, w.id, w.wait_mode)
                    if key not in best:
                        best[key] = w
                        order.append(key)
                    elif (w.wait_value or 0) > (best[key].wait_value or 0):
                        best[key] = w
                deduped = [best[k] for k in order]
                if len(deduped) != len(si.on_wait):
                    inst.sync_info = mybir.SyncInfo(
                        on_wait=deduped, on_update=si.on_update)


def _split_noops(nc):
    """Split multi-wait NoOps into single-wait chains (walrus's CTRL_NO
    struct carries very few sync commands)."""
    if getattr(nc, "_noops_split", False):
        return
    nc._noops_split = True
    split_id = [0]
    for fnn in nc.m.functions:
        for blk in fnn.blocks:
            out = []
            changed = False
            for inst in blk.instructions:
                si = inst.sync_info
                if (type(inst).__name__ == "InstNoOp" and si is not None
                        and len(si.on_wait) > 1):
                    changed = True
                    for w in si.on_wait[:-1]:
                        no = mybir.InstNoOp(
                            name=f"noop_waitsplit_{split_id[0]}",
                            text_hint="waitsplit")
                        split_id[0] += 1
                        no.engine = inst.engine
                        no.sync_info = mybir.SyncInfo(
                            on_wait=[w], on_update=[])
                        out.append(no)
                    inst.sync_info = mybir.SyncInfo(
                        on_wait=[si.on_wait[-1]], on_update=si.on_update)
                out.append(inst)
            if changed:
                blk.instructions = out


def _get_nc():
    global _NC_CACHE
    if _NC_CACHE is None:
        _NC_CACHE = _build_bass()
    return _NC_CACHE


def _make_in_maps(x, W1, W2, fw0):
    xs = np.ascontiguousarray(x[:, 0, :].astype(np.float32))       # [256, 128]
    xst = np.ascontiguousarray(xs.T)                                # [128, 256]
    W1 = np.asarray(W1, dtype=np.float32)
    W2 = np.asarray(W2, dtype=np.float32)
    fw0 = np.asarray(fw0, dtype=np.float32)

    # k-block-transposed W1: rows k*128+p, col c = W1[k*128+c, p]
    w1bt = np.ascontiguousarray(
        W1.reshape(KT, 128, IN).transpose(0, 2, 1).reshape(K2, 128))

    shared_rows = W2[MT_OWN * 128 * NCORES:, :]                     # [64, 16512]
    fw_shared = np.zeros(128, np.float32)
    fw_shared[0:HID] = fw0[MT_OWN * 128 * NCORES:]

    in_maps = []
    for c in range(NCORES):
        own = W2[c * 1024:(c + 1) * 1024, :]                        # [1024, 16512]
        w2c = np.concatenate([own, shared_rows], axis=0)            # [1088, 16512]
        wcomb = np.concatenate(
            [np.ascontiguousarray(w2c.T), w1bt], axis=1)            # [16512, 1216]
        wq = np.ascontiguousarray(
            wcomb.reshape(QT, T3, 128, WC).transpose(0, 2, 1, 3)
            .reshape(QT * 128, T3 * WC)).astype(np.float16)
        fw0_t = np.zeros((128, NT), np.float32)
        for m in range(MT_OWN):
            fw0_t[:, m] = fw0[c * 1024 + m * 128: c * 1024 + (m + 1) * 128]
        fw0_t[:, NT - 1] = fw_shared
        cst = np.zeros((128, B + NT), np.float32)
        cst[:, 0:B] = xst
        cst[:, B:B + NT] = fw0_t
        sel = np.zeros((128, MT_OWN * HID + 1), np.float16)
        for m in range(MT_OWN):
            sel[:, m * HID + MT_OWN * c + m] = 1.0
        sel[:, MT_OWN * HID] = 1.0
        in_maps.append({"wq": wq, "cst": cst, "sel": sel})
    return in_maps


def kernel(x, W1, W2, fw0, _trace=False, _tmpdir=None):
    nc = _get_nc()
    _split_noops(nc)
    in_maps = _make_in_maps(x, W1, W2, fw0)
    res = run_bass_kernel_spmd(
        nc, in_maps, core_ids=list(range(NCORES)),
        trace=_trace, tmpdir=_tmpdir,
    )
    preds = np.zeros((1, B), np.float64)
    for c in range(NCORES):
        preds += res.results[c]["pred"].astype(np.float64)
    out = preds.astype(np.float32).reshape(B, 1)
    if _trace:
        return out, res
    return out


# revision 36
# speedup vs baseline: 1.0141x; 1.0141x over previous
"""Trainium2 Bass kernel for nn_BruteForceUpdater (fp16 GEMM + stream scan).

Reference computation:
    xs = x[:, 0, :]                       # [256, 128]
    U  = (xs @ W1.T) @ W2.T               # [256, 8256]
    fw_{i+1} = sigmoid(10*(fw_i + U_i - 0.5))   (serial over batch)
    pred_i = fw2_i @ relu(fw1_i @ x_i)    # fw1 = fw[:8192].reshape(64,128)

Distribution over 8 NeuronCores (no collectives; host sums partials):
  * Core c owns fast-w1 row-tiles h in [8c, 8c+8) (1024 rows of W2); every
    core replicates the trailing 64 rows (the fast-w2 part) -> each core
    processes 1088 W2 rows = 9 output tiles (NT) of U.T.
  * Streamed tensor wq: k-blocked weights, fp16, 3 k-tiles per DMA
    ([128, 3*1216]): cols 0:1088 of each k-tile = the W2 slice
    pre-transposed, cols 1088:1216 = the k-block-transposed W1 chunk
    (T1 = W1 @ xs.T is produced on device 2 k-tiles ahead of the GEMM).
  * GEMM: fp16 operands (error ~5e-4 << the 2e-2 gate), fp32 psum
    accumulation over 129 k-tiles into U.T [9 tiles x 256].
  * Scan: the gain-10 sigmoid recurrence contracts hard, so the 256-step
    chain splits into 16 chunks; chunks >=2 rerun 24 warmup steps from
    state 0.5 (numpy-validated: reproduces the exact scan to ~5e-4).
    Chunks are merged into 4 independent streams (uniform col stride 48)
    so one DVE add + one ACT sigmoid advances 4-5 chunks at once; streams
    interleave round-robin, hiding the cross-engine dependency latency.
  * Prediction: prods fw1*x per m-tile (split Pool/DVE), 8 one-hot
    "sel" matmuls place each tile's column sums at psum partition 8c+m,
    one fused max(H,0)*fw2 op, one ones-matmul -> [1,256] partials.

Hand-rolled semaphores (one wait + one inc per instruction, walrus
limit); in-order engine queues carry the rest of the ordering.
"""
import os
import sys

sys.path.insert(0, "/opt/trn_rl_repo")

import numpy as np
from contextlib import ExitStack

import concourse.bass as bass
import concourse.tile as tile
from concourse import mybir
from concourse.bass_utils import run_bass_kernel_spmd

F32 = mybir.dt.float32
F16 = mybir.dt.float16
AF = mybir.ActivationFunctionType
ALU = mybir.AluOpType

IN = 128
HID = 64
NFW = IN * HID + HID          # 8256
B = 256
K2 = 2 * NFW                  # 16512
KT = K2 // 128                # 129 contraction tiles
NCORES = 8
MT_OWN = 8                    # full 128-row W2 tiles owned per core
NT = MT_OWN + 1               # + shared 64-row tile
MSL = MT_OWN * 128 + HID      # 1088 W2 rows handled per core
WC = MSL + 128                # k-tile width (W2 cols + W1 chunk)
GAIN, SHIFT = 10.0, 0.5

T3 = 3                        # k-tiles per streamed DMA
QT = KT // T3                 # 43 triple-DMAs
NQ = 8                        # triple ring slots
LOOKQ = 7                     # triples issued ahead
NT1 = 4                       # t1 ring slots

# scan streams: chunk 0 = cols 0:32 (exact, from fw0); chunks j=2..15 =
# cols 16j:16j+16 with 24 warmup steps from 0.5.  Streams group chunks
# at uniform stride 48 so one instruction advances a whole stream.
HWRM = 24
LC0 = 32                      # chunk-0 columns (rounds 0..31)
STREAMS = [[2, 5, 8, 11, 14], [3, 6, 9, 12, 15], [4, 7, 10, 13]]
ROUNDS = HWRM + 16            # 40
HP = 12                       # warmup rounds hidden under the GEMM tail
KP = 115                      # k-tiles in the partial-U warmup snapshot
HK0 = 119                     # k index where hidden rounds start issuing

_NC_CACHE = None


def _build_bass():
    nc = bass.Bass("TRN2", target_bir_lowering=False, debug=False)

    wq_d = nc.dram_tensor("wq", [QT * 128, T3 * WC], F16, kind="ExternalInput")
    cst_d = nc.dram_tensor("cst", [128, B + NT], F32, kind="ExternalInput")
    sel_d = nc.dram_tensor("sel", [128, MT_OWN * HID + 1], F16,
                           kind="ExternalInput")
    pred_d = nc.dram_tensor("pred", [1, B], F32, kind="ExternalOutput")
    dbg = bool(int(os.environ.get("KDBG", "0")))
    if dbg:
        u_dbg_d = nc.dram_tensor("u_dbg", [128, NT * B], F32,
                                 kind="ExternalOutput")
        fw_dbg_d = nc.dram_tensor("fw_dbg", [128, NT * B], F32,
                                  kind="ExternalOutput")
        q_dbg_d = nc.dram_tensor("q_dbg", [HID, B], F16,
                                 kind="ExternalOutput")

    with tile.TileContext(nc) as tc:
        with ExitStack() as ctx:
            const_pool = ctx.enter_context(tc.tile_pool(name="const", bufs=1))
            stream_pool = ctx.enter_context(tc.tile_pool(name="wcs", bufs=1))
            big_pool = ctx.enter_context(tc.tile_pool(name="big", bufs=1))

            cst = const_pool.tile([128, B + NT], F32)
            xst = cst[:, 0:B]
            fw0_t = cst[:, B:B + NT]
            sel = const_pool.tile([128, MT_OWN * HID + 1], F16)
            ones16 = sel[:, MT_OWN * HID:MT_OWN * HID + 1]
            xst_h = const_pool.tile([128, B], F16)
            bias_t = const_pool.tile([128, 1], F32)
            half_t = const_pool.tile([128, NT], F32)
            zf16 = const_pool.tile([128, 128], F16)

            wbuf = stream_pool.tile([128, NQ * T3 * WC], F16)  # stream ring
            t1r = big_pool.tile([128, NT1 * B], F16)           # T1.T ring
            u_sb = big_pool.tile([128, NT * B], F32)           # U.T slice
            up_sb = big_pool.tile([128, NT * B], F32)          # partial U.T
            fw_sb = big_pool.tile([128, NT * B], F32)          # fw history
            # per-stream add/sigmoid ping-pong + warmup state slots
            t_big = big_pool.tile([128, 2 * NT * (1 + 5 + 5 + 4)], F32)
            wsc = big_pool.tile([128, 2 * NT * (5 + 5 + 4)], F32)
            prod_h = big_pool.tile([128, MT_OWN * B], F16)     # fw1*x
            xst8 = big_pool.tile([128, MT_OWN * B], F16)       # x replicated
            q_sb = big_pool.tile([HID, B], F16)                # relu(H)*fw2
            pred_sb = big_pool.tile([1, B], F32)

            u_r = u_sb[:].rearrange("p (m i) -> p m i", m=NT)
            up_r = up_sb[:].rearrange("p (m i) -> p m i", m=NT)
            fw_r = fw_sb[:].rearrange("p (m i) -> p m i", m=NT)

            def qslot(q):
                s = q % NQ
                return wbuf[:, s * T3 * WC:(s + 1) * T3 * WC]

            def wslot(k):
                s = (k // T3) % NQ
                off = (s * T3 + k % T3) * WC
                return wbuf[:, off:off + WC]

            def t1slot(j):
                s = j % NT1
                return t1r[:, s * B:(s + 1) * B]

            # sub-bank psum packing for the 9 accumulating U.T tiles
            tile_bytes = B * 4
            bankof = [m * tile_bytes // 2048 for m in range(NT)]
            m_first = [m == 0 or bankof[m] != bankof[m - 1] for m in range(NT)]
            m_last = [m == NT - 1 or bankof[m] != bankof[m + 1] for m in range(NT)]

            csem = nc.alloc_semaphore("csem")
            dsem = [nc.alloc_semaphore(f"dsem{s}") for s in range(NQ)]
            tmm_sem = nc.alloc_semaphore("tmm")
            cp_sem = nc.alloc_semaphore("cp")
            pe_sem = nc.alloc_semaphore("pe")
            sv = nc.alloc_semaphore("sv")     # DVE progress
            sa = nc.alloc_semaphore("sa")     # ACT progress
            pl = nc.alloc_semaphore("pl")     # Pool progress
            pp = nc.alloc_semaphore("pp")     # PE pred progress
            ups = nc.alloc_semaphore("ups")   # partial-U snapshot done
            dsm = nc.alloc_semaphore("dsm")

            with tc.tile_pool(name="pt1", bufs=1, space="PSUM") as pt_pool, \
                 tc.tile_pool(name="pu", bufs=1, space="PSUM") as pu_pool:
                pt = pt_pool.tile([128, 1024], F32)
                psum_u = pu_pool.tile([128, NT * B], F32)
                pu_r = psum_u[:].rearrange("p (m i) -> p m i", m=NT)

                with tc.tile_critical():
                    svc = [0]                 # sv value tracker

                    def dve_inc(inst):
                        inst.then_inc(sv, 1)
                        svc[0] += 1
                        return svc[0]

                    # constants: cst rides the fast Sync hardware queue (a
                    # software-queue DMA costs ~5us completion latency and
                    # gates the whole startup); sel stays on the Pool queue
                    # (only needed post-scan).
                    nc.sync.dma_start(cst[:], cst_d[:, :]).then_inc(csem, 16)
                    nc.gpsimd.dma_start(sel[:], sel_d[:, :]).then_inc(csem, 16)
                    nc.vector.memset(bias_t[:], -GAIN * SHIFT)
                    dve_inc(nc.vector.memset(half_t[:], 0.5))
                    dve_inc(nc.vector.memset(zf16[:], 0.0))
                    cxr = nc.vector.tensor_copy(xst_h[:], xst)
                    cxr._wait_ge(csem, 16)
                    v_xr = dve_inc(cxr)

                    def dma_q(q):
                        d = nc.sync.dma_start(
                            qslot(q), wq_d[q * 128:(q + 1) * 128, :])
                        if q >= NQ:
                            d._wait_ge(pe_sem, T3 * (q - NQ) + T3)
                        d.then_inc(dsem[q % NQ], 16)

                    def t1_mm(j):
                        mm = nc.tensor.matmul(
                            pt[:, (j % 2) * 512:(j % 2) * 512 + B],
                            wslot(j)[:, MSL:WC], xst_h[:],
                            start=True, stop=True)
                        q = j // T3
                        mm._wait_ge(dsem[q % NQ], 16 * (q // NQ + 1))
                        mm.then_inc(tmm_sem, 1)

                    def t1_copy(j):
                        cp = nc.vector.tensor_copy(
                            t1slot(j), pt[:, (j % 2) * 512:(j % 2) * 512 + B])
                        cp._wait_ge(tmm_sem, j + 1)
                        cp.then_inc(cp_sem, 1)

                    # PE warmup (pulls the const prep tick into PE's clock)
                    zmm = nc.tensor.matmul(pt[:, 0:B], zf16[:], xst_h[:],
                                           start=True, stop=True)
                    zmm._wait_ge(sv, v_xr)

                    for q in range(LOOKQ):
                        dma_q(q)
                    for j in range(2):
                        t1_mm(j)
                        t1_copy(j)

                    # ---- scan stream descriptors (the GEMM-hidden warmup
                    # rounds are emitted inside the k-loop below) ----
                    sdesc = []      # (i0_base, n, t_off, w_off)
                    t_off = 0
                    w_off = 0
                    sdesc.append((0, 1, t_off, None))
                    t_off += 2 * NT
                    for s in STREAMS:
                        sdesc.append((16 * s[0] - HWRM, len(s), t_off, w_off))
                        t_off += 2 * NT * len(s)
                        w_off += 2 * NT * len(s)

                    def tview(si, r):
                        i0, n, to, _ = sdesc[si]
                        off = to + (r % 2) * NT * n
                        return t_big[:, off:off + NT * n].rearrange(
                            "p (m t) -> p m t", t=n)

                    def wview(si, r):
                        i0, n, _, wo = sdesc[si]
                        off = wo + (r % 2) * NT * n
                        return wsc[:, off:off + NT * n].rearrange(
                            "p (m t) -> p m t", t=n)

                    def fview(ap, si, r):
                        i0, n, _, _ = sdesc[si]
                        i = i0 + r
                        return ap[:, :, i:i + 48 * (n - 1) + 1:48]

                    v_add = {}
                    a_idx = {}
                    a_cnt = [0]
                    v_ucpa = [None]           # sa slot of the exact-U copy

                    def emit_add(si, r):
                        i0, n, _, _ = sdesc[si]
                        if r == 0:
                            prev = (fw0_t[:, :, None] if si == 0
                                    else half_t[:, :, None]).broadcast_to(
                                        (128, NT, n))
                        elif si > 0 and r - 1 < HWRM:
                            prev = wview(si, r - 1)
                        else:
                            prev = fview(fw_r, si, r - 1)
                        usrc = up_r if (si > 0 and r < HP) else u_r
                        add = nc.vector.tensor_add(
                            tview(si, r), prev, fview(usrc, si, r))
                        if (si > 0 and r == HP) or (si == 0 and r == 0):
                            # first exact-U read; also subsumes the hidden
                            # sigmoid of round HP-1 (ACT is in-order)
                            add._wait_ge(sa, v_ucpa[0])
                        elif r == 0:
                            add._wait_ge(ups, 1)  # partial-U copy done
                        else:
                            add._wait_ge(sa, a_idx[(si, r - 1)])
                        v_add[(si, r)] = dve_inc(add)

                    def emit_act(si, r):
                        out = (wview(si, r) if si > 0 and r < HWRM
                               else fview(fw_r, si, r))
                        act = nc.scalar.activation(
                            out, tview(si, r), AF.Sigmoid,
                            bias=bias_t[:], scale=GAIN)
                        act._wait_ge(sv, v_add[(si, r)])
                        act.then_inc(sa, 1)
                        a_cnt[0] += 1
                        a_idx[(si, r)] = a_cnt[0]

                    # partial-U snapshot: ACT (idle during the GEMM) copies
                    # psum mid-accumulation; values hold >= KP k-tiles,
                    # plenty for the contraction-dominated warmup steps
                    # NOTE: an engine reading PSUM while the PE's
                    # accumulation group is open faults the device, so the
                    # k-loop below parks the PE on a NOP (ups) for the ~2us
                    # this copy takes.
                    upc = nc.scalar.activation(up_sb[:], psum_u[:], AF.Copy)
                    upc._wait_ge(pe_sem, KP)
                    upc.then_inc(ups, 1)

                    for k in range(KT):
                        if k == KP:
                            nc.tensor.nop()._wait_ge(ups, 1)
                        if k % T3 == 0 and k // T3 + LOOKQ < QT:
                            dma_q(k // T3 + LOOKQ)
                        for m in range(NT):
                            mm = nc.tensor.matmul(
                                psum_u[:, m * B:(m + 1) * B],
                                wslot(k)[:, m * 128:(m + 1) * 128],
                                t1slot(k),
                                start=(k == 0 and m_first[m]),
                                stop=(k == KT - 1 and m_last[m]),
                            )
                            if m == 0:
                                mm._wait_ge(cp_sem, k + 1)
                            if m == NT - 1:
                                mm.then_inc(pe_sem, 1)
                        if k + 2 < KT:
                            t1_mm(k + 2)
                            t1_copy(k + 2)
                        if k in (96, 100, 104, 108):
                            # replicate x for the prod ops (DVE slack
                            # swallows the 0.55us pieces)
                            b4 = (k - 96) // 4 * 2
                            xc = nc.vector.tensor_copy(
                                xst8[:, b4 * B:(b4 + 2) * B].rearrange(
                                    "p (m b) -> p m b", m=2),
                                xst_h[:, None, :].broadcast_to((128, 2, B)))
                            dve_inc(xc)
                        if HK0 <= k < HK0 + HP - 2:
                            j = k - HK0
                            for si in (1, 2, 3):
                                emit_add(si, j)
                                emit_act(si, j)
                    for j in (HP - 2, HP - 1):
                        for si in (1, 2, 3):
                            emit_add(si, j)
                            emit_act(si, j)

                    # gate: sel-DMA arrival folded into the DVE stream here
                    # (long after issue; everything downstream inherits it)
                    g = nc.vector.memset(t_big[0:1, 0:1], 0.0)
                    g._wait_ge(csem, 32)
                    dve_inc(g)

                    # exact psum U -> sbuf, split DVE/ACT to halve the stall
                    ucp_d = nc.vector.tensor_copy(
                        u_sb[:, 0:NT * B // 2], psum_u[:, 0:NT * B // 2])
                    ucp_d._wait_ge(pe_sem, KT)
                    dve_inc(ucp_d)
                    ucp_a = nc.scalar.activation(
                        u_sb[:, NT * B // 2:], psum_u[:, NT * B // 2:],
                        AF.Copy)
                    ucp_a._wait_ge(pe_sem, KT)
                    ucp_a.then_inc(sa, 1)
                    a_cnt[0] += 1
                    v_ucpa[0] = a_cnt[0]

                    # ---- visible scan rounds + hidden per-round prods ----
                    # chunks 2..15 sit at uniform column stride 16, so each
                    # round's 14 fresh columns are one strided Pool op; the
                    # 8-col blocks of chunk 0 batch the same way.
                    prod_r = prod_h[:].rearrange("p (m b) -> p m b", m=MT_OWN)
                    xst8_r = xst8[:].rearrange("p (m b) -> p m b", m=MT_OWN)
                    pool_plan = {}           # slot -> list of ops
                    for r in range(24, ROUNDS):
                        pool_plan.setdefault(r - HP + 1, []).append(("rp", r))
                    for b in range(4):
                        pool_plan.setdefault(8 * b + 8, []).append(("s0", b))
                    v_rp_done = [None]
                    v_prods = [None]

                    def emit_prod(kind, arg):
                        if kind == "rp":
                            r = arg
                            i2 = 32 + (r - 24)
                            sl = slice(i2, i2 + 16 * 13 + 1, 16)
                            wait = a_idx[(3, r)]
                        else:
                            b = arg
                            sl = slice(8 * b, 8 * b + 8)
                            wait = a_idx[(0, 8 * b + 7)]
                        pr = nc.vector.tensor_mul(
                            prod_r[:, :, sl], fw_r[:, 0:MT_OWN, sl],
                            xst8_r[:, :, sl])
                        pr._wait_ge(sa, wait)
                        val = dve_inc(pr)
                        if kind == "rp" and arg == ROUNDS - 1:
                            v_rp_done[0] = val
                        v_prods[0] = val

                    plan_done = set()
                    for v in range(LC0 + 1):
                        if v < LC0:
                            emit_add(0, v)
                            emit_act(0, v)
                            r = HP + v
                            if r < ROUNDS:
                                for si in (1, 2, 3):
                                    emit_add(si, r)
                                    emit_act(si, r)
                        for kind, arg in pool_plan.get(v, []):
                            emit_prod(kind, arg)
                    a_total = a_cnt[0]

                    # ---- prediction: sel matmuls place H[m,:] at psum
                    # partition 8*core+m; cols 32:256 run during chunk-0's
                    # tail rounds, cols 0:32 right after its last sigmoid
                    for idx, m in enumerate(range(MT_OWN)):
                        hm = nc.tensor.matmul(
                            pt[0:HID, 32:B], sel[:, m * HID:(m + 1) * HID],
                            prod_h[:, m * B + 32:m * B + B],
                            start=(idx == 0), stop=False)
                        if idx == 0:
                            hm._wait_ge(sv, v_rp_done[0])
                    for idx, m in enumerate(range(MT_OWN)):
                        hm = nc.tensor.matmul(
                            pt[0:HID, 0:32], sel[:, m * HID:(m + 1) * HID],
                            prod_h[:, m * B:m * B + 32],
                            start=False, stop=(idx == MT_OWN - 1))
                        if idx == 0:
                            hm._wait_ge(sv, v_prods[0])
                        if idx == MT_OWN - 1:
                            hm.then_inc(pp, 1)

                    # q = max(H,0) * fw2  (fused, psum+sbuf -> sbuf fp16)
                    stt = nc.vector.scalar_tensor_tensor(
                        q_sb[:, :], pt[0:HID, 0:B], 0.0,
                        fw_r[0:HID, NT - 1, :], op0=ALU.max, op1=ALU.mult)
                    stt._wait_ge(pp, 1)
                    v_stt = dve_inc(stt)

                    pmm = nc.tensor.matmul(
                        psum_u[0:1, 0:B], ones16[0:HID, :], q_sb[:, :],
                        start=True, stop=True)
                    pmm._wait_ge(sv, v_stt)
                    pmm.then_inc(pp, 1)

                    cpd = nc.vector.tensor_copy(pred_sb[:], psum_u[0:1, 0:B])
                    cpd._wait_ge(pp, 2)
                    v_out = dve_inc(cpd)

                    dout = nc.sync.dma_start(pred_d[:, :], pred_sb[:])
                    dout._wait_ge(sv, v_out)
                    dout.then_inc(dsm, 16)
                    if dbg:
                        du = nc.sync.dma_start(u_dbg_d[:, :], u_sb[:])
                        du._wait_ge(sv, v_out)
                        du.then_inc(dsm, 16)
                        df = nc.sync.dma_start(fw_dbg_d[:, :], fw_sb[:])
                        df._wait_ge(sv, v_out)
                        df.then_inc(dsm, 16)
                        dq = nc.sync.dma_start(q_dbg_d[:, :], q_sb[:])
                        dq._wait_ge(sv, v_out)
                        dq.then_inc(dsm, 16)

    _dedupe_waits(nc)
    return nc


def _dedupe_waits(nc):
    """Collapse duplicate semaphore waits the framework occasionally emits
    (walrus allows very few sync commands per instruction)."""
    for fnn in nc.m.functions:
        for blk in fnn.blocks:
            for inst in blk.instructions:
                si = inst.sync_info
                if si is None or not si.on_wait or len(si.on_wait) < 2:
                    continue
                best = {}
                order = []
                for w in si.on_wait:
                    if w.wait_reg is not None or w.wait_mode != "sem-ge-imm":
                        key = ("raw", id(w))
                    else:
                        key = (w.sync_type, w.id, w.wait_mode)
                    if key not in best:
                        best[key] = w
                        order.append(key)
                    elif (w.wait_value or 0) > (best[key].wait_value or 0):
                        best[key] = w
                deduped = [best[k] for k in order]
                if len(deduped) != len(si.on_wait):
                    inst.sync_info = mybir.SyncInfo(
                        on_wait=deduped, on_update=si.on_update)


def _split_noops(nc):
    """Split multi-wait NoOps into single-wait chains (walrus's CTRL_NO
    struct carries very few sync commands)."""
    if getattr(nc, "_noops_split", False):
        return
    nc._noops_split = True
    split_id = [0]
    for fnn in nc.m.functions:
        for blk in fnn.blocks:
            out = []
            changed = False
            for inst in blk.instructions:
                si = inst.sync_info
                if (type(inst).__name__ == "InstNoOp" and si is not None
                        and len(si.on_wait) > 1):
                    changed = True
                    for w in si.on_wait[:-1]:
                        no = mybir.InstNoOp(
                            name=f"noop_waitsplit_{split_id[0]}",
                            text_hint="waitsplit")
                        split_id[0] += 1
                        no.engine = inst.engine
                        no.sync_info = mybir.SyncInfo(
                            on_wait=[w], on_update=[])
                        out.append(no)
                    inst.sync_info = mybir.SyncInfo(
                        on_wait=[si.on_wait[-1]], on_update=si.on_update)
                out.append(inst)
            if changed:
                blk.instructions = out


def _get_nc():
    global _NC_CACHE
    if _NC_CACHE is None:
        _NC_CACHE = _build_bass()
    return _NC_CACHE


def _make_in_maps(x, W1, W2, fw0):
    xs = np.ascontiguousarray(x[:, 0, :].astype(np.float32))       # [256, 128]
    xst = np.ascontiguousarray(xs.T)                                # [128, 256]
    W1 = np.asarray(W1, dtype=np.float32)
    W2 = np.asarray(W2, dtype=np.float32)
    fw0 = np.asarray(fw0, dtype=np.float32)

    # k-block-transposed W1: rows k*128+p, col c = W1[k*128+c, p]
    w1bt = np.ascontiguousarray(
        W1.reshape(KT, 128, IN).transpose(0, 2, 1).reshape(K2, 128))

    shared_rows = W2[MT_OWN * 128 * NCORES:, :]                     # [64, 16512]
    fw_shared = np.zeros(128, np.float32)
    fw_shared[0:HID] = fw0[MT_OWN * 128 * NCORES:]

    in_maps = []
    for c in range(NCORES):
        own = W2[c * 1024:(c + 1) * 1024, :]                        # [1024, 16512]
        w2c = np.concatenate([own, shared_rows], axis=0)            # [1088, 16512]
        wcomb = np.concatenate(
            [np.ascontiguousarray(w2c.T), w1bt], axis=1)            # [16512, 1216]
        wq = np.ascontiguousarray(
            wcomb.reshape(QT, T3, 128, WC).transpose(0, 2, 1, 3)
            .reshape(QT * 128, T3 * WC)).astype(np.float16)
        fw0_t = np.zeros((128, NT), np.float32)
        for m in range(MT_OWN):
            fw0_t[:, m] = fw0[c * 1024 + m * 128: c * 1024 + (m + 1) * 128]
        fw0_t[:, NT - 1] = fw_shared
        cst = np.zeros((128, B + NT), np.float32)
        cst[:, 0:B] = xst
        cst[:, B:B + NT] = fw0_t
        sel = np.zeros((128, MT_OWN * HID + 1), np.float16)
        for m in range(MT_OWN):
            sel[:, m * HID + MT_OWN * c + m] = 1.0
        sel[:, MT_OWN * HID] = 1.0
        in_maps.append({"wq": wq, "cst": cst, "sel": sel})
    return in_maps


def kernel(x, W1, W2, fw0, _trace=False, _tmpdir=None):
    nc = _get_nc()
    _split_noops(nc)
    in_maps = _make_in_maps(x, W1, W2, fw0)
    res = run_bass_kernel_spmd(
        nc, in_maps, core_ids=list(range(NCORES)),
        trace=_trace, tmpdir=_tmpdir,
    )
    preds = np.zeros((1, B), np.float64)
    for c in range(NCORES):
        preds += res.results[c]["pred"].astype(np.float64)
    out = preds.astype(np.float32).reshape(B, 1)
    if _trace:
        return out, res
    return out


# revision 42
# speedup vs baseline: 1.0252x; 1.0109x over previous
"""Trainium2 Bass kernel for nn_BruteForceUpdater (fp16 GEMM + stream scan).

Reference computation:
    xs = x[:, 0, :]                       # [256, 128]
    U  = (xs @ W1.T) @ W2.T               # [256, 8256]
    fw_{i+1} = sigmoid(10*(fw_i + U_i - 0.5))   (serial over batch)
    pred_i = fw2_i @ relu(fw1_i @ x_i)    # fw1 = fw[:8192].reshape(64,128)

Distribution over 8 NeuronCores (no collectives; host sums partials):
  * Core c owns fast-w1 row-tiles h in [8c, 8c+8) (1024 rows of W2); every
    core replicates the trailing 64 rows (the fast-w2 part) -> each core
    processes 1088 W2 rows = 9 output tiles (NT) of U.T.
  * Streamed tensor wq: k-blocked weights, fp16, 3 k-tiles per DMA
    ([128, 3*1216]): cols 0:1088 of each k-tile = the W2 slice
    pre-transposed, cols 1088:1216 = the k-block-transposed W1 chunk
    (T1 = W1 @ xs.T is produced on device 2 k-tiles ahead of the GEMM).
  * GEMM: fp16 operands (error ~5e-4 << the 2e-2 gate), fp32 psum
    accumulation over 129 k-tiles into U.T [9 tiles x 256].
  * Scan: the gain-10 sigmoid recurrence contracts hard, so the 256-step
    chain splits into 16 chunks; chunks >=2 rerun 24 warmup steps from
    state 0.5 (numpy-validated: reproduces the exact scan to ~5e-4).
    Chunks are merged into 4 independent streams (uniform col stride 48)
    so one DVE add + one ACT sigmoid advances 4-5 chunks at once; streams
    interleave round-robin, hiding the cross-engine dependency latency.
  * Prediction: prods fw1*x per m-tile (split Pool/DVE), 8 one-hot
    "sel" matmuls place each tile's column sums at psum partition 8c+m,
    one fused max(H,0)*fw2 op, one ones-matmul -> [1,256] partials.

Hand-rolled semaphores (one wait + one inc per instruction, walrus
limit); in-order engine queues carry the rest of the ordering.
"""
import os
import sys

sys.path.insert(0, "/opt/trn_rl_repo")

import numpy as np
from contextlib import ExitStack

import concourse.bass as bass
import concourse.tile as tile
from concourse import mybir
from concourse.bass_utils import run_bass_kernel_spmd

F32 = mybir.dt.float32
F16 = mybir.dt.float16
AF = mybir.ActivationFunctionType
ALU = mybir.AluOpType

IN = 128
HID = 64
NFW = IN * HID + HID          # 8256
B = 256
K2 = 2 * NFW                  # 16512
KT = K2 // 128                # 129 contraction tiles
NCORES = 8
MT_OWN = 8                    # full 128-row W2 tiles owned per core
NT = MT_OWN + 1               # + shared 64-row tile
MSL = MT_OWN * 128 + HID      # 1088 W2 rows handled per core
WC = MSL + 128                # k-tile width (W2 cols + W1 chunk)
GAIN, SHIFT = 10.0, 0.5

T3 = 3                        # k-tiles per streamed DMA
QT = KT // T3                 # 43 triple-DMAs
NQ = 8                        # triple ring slots
LOOKQ = 7                     # triples issued ahead
NT1 = 4                       # t1 ring slots

# scan streams: chunk 0 = cols 0:32 (exact, from fw0); chunks j=2..15 =
# cols 16j:16j+16 with 24 warmup steps from 0.5.  Streams group chunks
# at uniform stride 48 so one instruction advances a whole stream.
HWRM = 24
LC0 = 32                      # chunk-0 columns (rounds 0..31)
STREAMS = [[2, 5, 8, 11, 14], [3, 6, 9, 12, 15], [4, 7, 10, 13]]
ROUNDS = HWRM + 16            # 40
HP = 12                       # warmup rounds hidden under the GEMM tail
KP = 100                      # k-tiles in the partial-U warmup snapshot
HK0 = 104                     # k index where hidden rounds start issuing

_NC_CACHE = None


def _build_bass():
    nc = bass.Bass("TRN2", target_bir_lowering=False, debug=False)

    wq_d = nc.dram_tensor("wq", [QT * 128, T3 * WC], F16, kind="ExternalInput")
    cst_d = nc.dram_tensor("cst", [128, B + NT], F32, kind="ExternalInput")
    sel_d = nc.dram_tensor("sel", [128, MT_OWN * HID + 1], F16,
                           kind="ExternalInput")
    pred_d = nc.dram_tensor("pred", [1, B], F32, kind="ExternalOutput")
    dbg = bool(int(os.environ.get("KDBG", "0")))
    if dbg:
        u_dbg_d = nc.dram_tensor("u_dbg", [128, NT * B], F32,
                                 kind="ExternalOutput")
        fw_dbg_d = nc.dram_tensor("fw_dbg", [128, NT * B], F32,
                                  kind="ExternalOutput")
        q_dbg_d = nc.dram_tensor("q_dbg", [HID, B], F16,
                                 kind="ExternalOutput")

    with tile.TileContext(nc) as tc:
        with ExitStack() as ctx:
            const_pool = ctx.enter_context(tc.tile_pool(name="const", bufs=1))
            stream_pool = ctx.enter_context(tc.tile_pool(name="wcs", bufs=1))
            big_pool = ctx.enter_context(tc.tile_pool(name="big", bufs=1))

            cst = const_pool.tile([128, B + NT], F32)
            xst = cst[:, 0:B]
            fw0_t = cst[:, B:B + NT]
            sel = const_pool.tile([128, MT_OWN * HID + 1], F16)
            ones16 = sel[:, MT_OWN * HID:MT_OWN * HID + 1]
            xst_h = const_pool.tile([128, B], F16)
            bias_t = const_pool.tile([128, 1], F32)
            half_t = const_pool.tile([128, NT], F32)
            zf16 = const_pool.tile([128, 128], F16)

            wbuf = stream_pool.tile([128, NQ * T3 * WC], F16)  # stream ring
            t1r = big_pool.tile([128, NT1 * B], F16)           # T1.T ring
            u_sb = big_pool.tile([128, NT * B], F32)           # U.T slice
            up_sb = big_pool.tile([128, NT * B], F32)          # partial U.T
            fw_sb = big_pool.tile([128, NT * B], F32)          # fw history
            # per-stream add/sigmoid ping-pong + warmup state slots
            t_big = big_pool.tile([128, 2 * NT * (1 + 5 + 5 + 4)], F32)
            wsc = big_pool.tile([128, 2 * NT * (5 + 5 + 4)], F32)
            prod_h = big_pool.tile([128, MT_OWN * B], F16)     # fw1*x
            xst8 = big_pool.tile([128, MT_OWN * B], F16)       # x replicated
            q_sb = big_pool.tile([HID, B], F16)                # relu(H)*fw2
            pred_sb = big_pool.tile([1, B], F32)

            u_r = u_sb[:].rearrange("p (m i) -> p m i", m=NT)
            up_r = up_sb[:].rearrange("p (m i) -> p m i", m=NT)
            fw_r = fw_sb[:].rearrange("p (m i) -> p m i", m=NT)

            def qslot(q):
                s = q % NQ
                return wbuf[:, s * T3 * WC:(s + 1) * T3 * WC]

            def wslot(k):
                s = (k // T3) % NQ
                off = (s * T3 + k % T3) * WC
                return wbuf[:, off:off + WC]

            def t1slot(j):
                s = j % NT1
                return t1r[:, s * B:(s + 1) * B]

            # sub-bank psum packing for the 9 accumulating U.T tiles
            tile_bytes = B * 4
            bankof = [m * tile_bytes // 2048 for m in range(NT)]
            m_first = [m == 0 or bankof[m] != bankof[m - 1] for m in range(NT)]
            m_last = [m == NT - 1 or bankof[m] != bankof[m + 1] for m in range(NT)]

            csem = nc.alloc_semaphore("csem")
            dsem = [nc.alloc_semaphore(f"dsem{s}") for s in range(NQ)]
            tmm_sem = nc.alloc_semaphore("tmm")
            cp_sem = nc.alloc_semaphore("cp")
            pe_sem = nc.alloc_semaphore("pe")
            sv = nc.alloc_semaphore("sv")     # DVE progress
            sa = nc.alloc_semaphore("sa")     # ACT progress
            pl = nc.alloc_semaphore("pl")     # Pool progress
            pp = nc.alloc_semaphore("pp")     # PE pred progress
            ups = nc.alloc_semaphore("ups")   # partial-U snapshot done
            dsm = nc.alloc_semaphore("dsm")

            with tc.tile_pool(name="pt1", bufs=1, space="PSUM") as pt_pool, \
                 tc.tile_pool(name="pu", bufs=1, space="PSUM") as pu_pool:
                pt = pt_pool.tile([128, 1024], F32)
                psum_u = pu_pool.tile([128, NT * B], F32)
                pu_r = psum_u[:].rearrange("p (m i) -> p m i", m=NT)

                with tc.tile_critical():
                    svc = [0]                 # sv value tracker

                    def dve_inc(inst):
                        inst.then_inc(sv, 1)
                        svc[0] += 1
                        return svc[0]

                    # constants: cst rides the fast Sync hardware queue (a
                    # software-queue DMA costs ~5us completion latency and
                    # gates the whole startup); sel stays on the Pool queue
                    # (only needed post-scan).
                    nc.sync.dma_start(cst[:], cst_d[:, :]).then_inc(csem, 16)
                    nc.gpsimd.dma_start(sel[:], sel_d[:, :]).then_inc(csem, 16)
                    nc.vector.memset(bias_t[:], -GAIN * SHIFT)
                    dve_inc(nc.vector.memset(half_t[:], 0.5))
                    dve_inc(nc.vector.memset(zf16[:], 0.0))
                    cxr = nc.vector.tensor_copy(xst_h[:], xst)
                    cxr._wait_ge(csem, 16)
                    v_xr = dve_inc(cxr)

                    def dma_q(q):
                        d = nc.sync.dma_start(
                            qslot(q), wq_d[q * 128:(q + 1) * 128, :])
                        if q >= NQ:
                            d._wait_ge(pe_sem, T3 * (q - NQ) + T3)
                        d.then_inc(dsem[q % NQ], 16)

                    def t1_mm(j):
                        mm = nc.tensor.matmul(
                            pt[:, (j % 2) * 512:(j % 2) * 512 + B],
                            wslot(j)[:, MSL:WC], xst_h[:],
                            start=True, stop=True)
                        q = j // T3
                        mm._wait_ge(dsem[q % NQ], 16 * (q // NQ + 1))
                        mm.then_inc(tmm_sem, 1)

                    def t1_copy(j):
                        cp = nc.vector.tensor_copy(
                            t1slot(j), pt[:, (j % 2) * 512:(j % 2) * 512 + B])
                        cp._wait_ge(tmm_sem, j + 1)
                        cp.then_inc(cp_sem, 1)

                    # PE warmup (pulls the const prep tick into PE's clock)
                    zmm = nc.tensor.matmul(pt[:, 0:B], zf16[:], xst_h[:],
                                           start=True, stop=True)
                    zmm._wait_ge(sv, v_xr)

                    for q in range(LOOKQ):
                        dma_q(q)
                    for j in range(2):
                        t1_mm(j)
                        t1_copy(j)

                    # ---- scan stream descriptors (the GEMM-hidden warmup
                    # rounds are emitted inside the k-loop below) ----
                    sdesc = []      # (i0_base, n, t_off, w_off)
                    t_off = 0
                    w_off = 0
                    sdesc.append((0, 1, t_off, None))
                    t_off += 2 * NT
                    for s in STREAMS:
                        sdesc.append((16 * s[0] - HWRM, len(s), t_off, w_off))
                        t_off += 2 * NT * len(s)
                        w_off += 2 * NT * len(s)

                    def tview(si, r):
                        i0, n, to, _ = sdesc[si]
                        off = to + (r % 2) * NT * n
                        return t_big[:, off:off + NT * n].rearrange(
                            "p (m t) -> p m t", t=n)

                    def wview(si, r):
                        i0, n, _, wo = sdesc[si]
                        off = wo + (r % 2) * NT * n
                        return wsc[:, off:off + NT * n].rearrange(
                            "p (m t) -> p m t", t=n)

                    def fview(ap, si, r):
                        i0, n, _, _ = sdesc[si]
                        i = i0 + r
                        return ap[:, :, i:i + 48 * (n - 1) + 1:48]

                    v_add = {}
                    a_idx = {}
                    a_cnt = [0]
                    v_ucpa = [None]           # sa slot of the exact-U copy

                    def emit_add(si, r):
                        i0, n, _, _ = sdesc[si]
                        if r == 0:
                            prev = (fw0_t[:, :, None] if si == 0
                                    else half_t[:, :, None]).broadcast_to(
                                        (128, NT, n))
                        elif si > 0 and r - 1 < HWRM:
                            prev = wview(si, r - 1)
                        else:
                            prev = fview(fw_r, si, r - 1)
                        usrc = up_r if (si > 0 and r < HP) else u_r
                        add = nc.vector.tensor_add(
                            tview(si, r), prev, fview(usrc, si, r))
                        if (si > 0 and r == HP) or (si == 0 and r == 0):
                            # first exact-U read; also subsumes the hidden
                            # sigmoid of round HP-1 (ACT is in-order)
                            add._wait_ge(sa, v_ucpa[0])
                        elif r == 0:
                            add._wait_ge(ups, 1)  # partial-U copy done
                        else:
                            add._wait_ge(sa, a_idx[(si, r - 1)])
                        v_add[(si, r)] = dve_inc(add)

                    def emit_act(si, r):
                        out = (wview(si, r) if si > 0 and r < HWRM
                               else fview(fw_r, si, r))
                        act = nc.scalar.activation(
                            out, tview(si, r), AF.Sigmoid,
                            bias=bias_t[:], scale=GAIN)
                        act._wait_ge(sv, v_add[(si, r)])
                        act.then_inc(sa, 1)
                        a_cnt[0] += 1
                        a_idx[(si, r)] = a_cnt[0]

                    # partial-U snapshot: ACT (idle during the GEMM) copies
                    # psum mid-accumulation; values hold >= KP k-tiles,
                    # plenty for the contraction-dominated warmup steps
                    for k in range(KT):
                        if k == KP:
                            # partial-U snapshot: PSUM reads against the
                            # PE's open accumulation group are only safe
                            # from ACT with the PE parked on a NOP (DVE
                            # reads race, a running PE faults).
                            nc.tensor.nop()._wait_ge(ups, 1)
                            upa = nc.scalar.activation(
                                up_sb[:], psum_u[:], AF.Copy)
                            upa._wait_ge(pe_sem, KP)
                            upa.then_inc(ups, 1)
                        if k % T3 == 0 and k // T3 + LOOKQ < QT:
                            dma_q(k // T3 + LOOKQ)
                        for m in range(NT):
                            mm = nc.tensor.matmul(
                                psum_u[:, m * B:(m + 1) * B],
                                wslot(k)[:, m * 128:(m + 1) * 128],
                                t1slot(k),
                                start=(k == 0 and m_first[m]),
                                stop=(k == KT - 1 and m_last[m]),
                            )
                            if m == 0:
                                mm._wait_ge(cp_sem, k + 1)
                            if m == NT - 1:
                                mm.then_inc(pe_sem, 1)
                        if k + 2 < KT:
                            t1_mm(k + 2)
                            t1_copy(k + 2)
                        if k in (96, 100, 104, 108):
                            # replicate x for the prod ops (DVE slack
                            # swallows the 0.55us pieces)
                            b4 = (k - 96) // 4 * 2
                            xc = nc.vector.tensor_copy(
                                xst8[:, b4 * B:(b4 + 2) * B].rearrange(
                                    "p (m b) -> p m b", m=2),
                                xst_h[:, None, :].broadcast_to((128, 2, B)))
                            dve_inc(xc)
                        if HK0 <= k < HK0 + 2 * HP and (k - HK0) % 2 == 0:
                            j = (k - HK0) // 2
                            for si in (1, 2, 3):
                                emit_add(si, j)
                                emit_act(si, j)

                    # gate: sel-DMA arrival folded into the DVE stream here
                    # (long after issue; everything downstream inherits it)
                    g = nc.vector.memset(t_big[0:1, 0:1], 0.0)
                    g._wait_ge(csem, 32)
                    dve_inc(g)

                    # exact psum U -> sbuf, split DVE/ACT to halve the stall
                    ucp_d = nc.vector.tensor_copy(
                        u_sb[:, 0:NT * B // 2], psum_u[:, 0:NT * B // 2])
                    ucp_d._wait_ge(pe_sem, KT)
                    dve_inc(ucp_d)
                    ucp_a = nc.scalar.activation(
                        u_sb[:, NT * B // 2:], psum_u[:, NT * B // 2:],
                        AF.Copy)
                    ucp_a._wait_ge(pe_sem, KT)
                    ucp_a.then_inc(sa, 1)
                    a_cnt[0] += 1
                    v_ucpa[0] = a_cnt[0]

                    # ---- visible scan rounds + hidden per-round prods ----
                    # chunks 2..15 sit at uniform column stride 16, so each
                    # round's 14 fresh columns are one strided Pool op; the
                    # 8-col blocks of chunk 0 batch the same way.
                    prod_r = prod_h[:].rearrange("p (m b) -> p m b", m=MT_OWN)
                    xst8_r = xst8[:].rearrange("p (m b) -> p m b", m=MT_OWN)
                    pool_plan = {}           # slot -> list of ops
                    for r in range(24, ROUNDS):
                        pool_plan.setdefault(r - HP + 1, []).append(("rp", r))
                    for b in range(4):
                        pool_plan.setdefault(8 * b + 8, []).append(("s0", b))
                    v_rp_done = [None]
                    v_prods = [None]

                    def emit_prod(kind, arg):
                        if kind == "rp":
                            r = arg
                            i2 = 32 + (r - 24)
                            sl = slice(i2, i2 + 16 * 13 + 1, 16)
                            wait = a_idx[(3, r)]
                        else:
                            b = arg
                            sl = slice(8 * b, 8 * b + 8)
                            wait = a_idx[(0, 8 * b + 7)]
                        pr = nc.vector.tensor_mul(
                            prod_r[:, :, sl], fw_r[:, 0:MT_OWN, sl],
                            xst8_r[:, :, sl])
                        pr._wait_ge(sa, wait)
                        val = dve_inc(pr)
                        if kind == "rp" and arg == ROUNDS - 1:
                            v_rp_done[0] = val
                        v_prods[0] = val

                    plan_done = set()
                    for v in range(LC0 + 1):
                        if v < LC0:
                            emit_add(0, v)
                            emit_act(0, v)
                            r = HP + v
                            if r < ROUNDS:
                                for si in (1, 2, 3):
                                    emit_add(si, r)
                                    emit_act(si, r)
                        for kind, arg in pool_plan.get(v, []):
                            emit_prod(kind, arg)
                    a_total = a_cnt[0]

                    # ---- prediction: sel matmuls place H[m,:] at psum
                    # partition 8*core+m; cols 32:256 run during chunk-0's
                    # tail rounds, cols 0:32 right after its last sigmoid
                    for idx, m in enumerate(range(MT_OWN)):
                        hm = nc.tensor.matmul(
                            pt[0:HID, 32:B], sel[:, m * HID:(m + 1) * HID],
                            prod_h[:, m * B + 32:m * B + B],
                            start=(idx == 0), stop=False)
                        if idx == 0:
                            hm._wait_ge(sv, v_rp_done[0])
                    for idx, m in enumerate(range(MT_OWN)):
                        hm = nc.tensor.matmul(
                            pt[0:HID, 0:32], sel[:, m * HID:(m + 1) * HID],
                            prod_h[:, m * B:m * B + 32],
                            start=False, stop=(idx == MT_OWN - 1))
                        if idx == 0:
                            hm._wait_ge(sv, v_prods[0])
                        if idx == MT_OWN - 1:
                            hm.then_inc(pp, 1)

                    # q = max(H,0) * fw2  (fused, psum+sbuf -> sbuf fp16)
                    stt = nc.vector.scalar_tensor_tensor(
                        q_sb[:, :], pt[0:HID, 0:B], 0.0,
                        fw_r[0:HID, NT - 1, :], op0=ALU.max, op1=ALU.mult)
                    stt._wait_ge(pp, 1)
                    v_stt = dve_inc(stt)

                    pmm = nc.tensor.matmul(
                        psum_u[0:1, 0:B], ones16[0:HID, :], q_sb[:, :],
                        start=True, stop=True)
                    pmm._wait_ge(sv, v_stt)
                    pmm.then_inc(pp, 1)

                    cpd = nc.vector.tensor_copy(pred_sb[:], psum_u[0:1, 0:B])
                    cpd._wait_ge(pp, 2)
                    v_out = dve_inc(cpd)

                    dout = nc.sync.dma_start(pred_d[:, :], pred_sb[:])
                    dout._wait_ge(sv, v_out)
                    dout.then_inc(dsm, 16)
                    if dbg:
                        du = nc.sync.dma_start(u_dbg_d[:, :], u_sb[:])
                        du._wait_ge(sv, v_out)
                        du.then_inc(dsm, 16)
                        df = nc.sync.dma_start(fw_dbg_d[:, :], fw_sb[:])
                        df._wait_ge(sv, v_out)
                        df.then_inc(dsm, 16)
                        dq = nc.sync.dma_start(q_dbg_d[:, :], q_sb[:])
                        dq._wait_ge(sv, v_out)
                        dq.then_inc(dsm, 16)

    _dedupe_waits(nc)
    return nc


def _dedupe_waits(nc):
    """Collapse duplicate semaphore waits the framework occasionally emits
    (walrus allows very few sync commands per instruction)."""
    for fnn in nc.m.functions:
        for blk in fnn.blocks:
            for inst in blk.instructions:
                si = inst.sync_info
                if si is None or not si.on_wait or len(si.on_wait) < 2:
                    continue
                best = {}
                order = []
                for w in si.on_wait:
                    if w.wait_reg is not None or w.wait_mode != "sem-ge-imm":
                        key = ("raw", id(w))
                    else:
                        key = (w.sync_type, w.id, w.wait_mode)
                    if key not in best:
                        best[key] = w
                        order.append(key)
                    elif (w.wait_value or 0) > (best[key].wait_value or 0):
                        best[key] = w
                deduped = [best[k] for k in order]
                if len(deduped) != len(si.on_wait):
                    inst.sync_info = mybir.SyncInfo(
                        on_wait=deduped, on_update=si.on_update)


def _split_noops(nc):
    """Split multi-wait NoOps into single-wait chains (walrus's CTRL_NO
    struct carries very few sync commands)."""
    if getattr(nc, "_noops_split", False):
        return
    nc._noops_split = True
    split_id = [0]
    for fnn in nc.m.functions:
        for blk in fnn.blocks:
            out = []
            changed = False
            for inst in blk.instructions:
                si = inst.sync_info
                if (type(inst).__name__ == "InstNoOp" and si is not None
                        and len(si.on_wait) > 1):
                    changed = True
                    for w in si.on_wait[:-1]:
                        no = mybir.InstNoOp(
                            name=f"noop_waitsplit_{split_id[0]}",
                            text_hint="waitsplit")
                        split_id[0] += 1
                        no.engine = inst.engine
                        no.sync_info = mybir.SyncInfo(
                            on_wait=[w], on_update=[])
                        out.append(no)
                    inst.sync_info = mybir.SyncInfo(
                        on_wait=[si.on_wait[-1]], on_update=si.on_update)
                out.append(inst)
            if changed:
                blk.instructions = out


def _get_nc():
    global _NC_CACHE
    if _NC_CACHE is None:
        _NC_CACHE = _build_bass()
    return _NC_CACHE


def _make_in_maps(x, W1, W2, fw0):
    xs = np.ascontiguousarray(x[:, 0, :].astype(np.float32))       # [256, 128]
    xst = np.ascontiguousarray(xs.T)                                # [128, 256]
    W1 = np.asarray(W1, dtype=np.float32)
    W2 = np.asarray(W2, dtype=np.float32)
    fw0 = np.asarray(fw0, dtype=np.float32)

    # k-block-transposed W1: rows k*128+p, col c = W1[k*128+c, p]
    w1bt = np.ascontiguousarray(
        W1.reshape(KT, 128, IN).transpose(0, 2, 1).reshape(K2, 128))

    shared_rows = W2[MT_OWN * 128 * NCORES:, :]                     # [64, 16512]
    fw_shared = np.zeros(128, np.float32)
    fw_shared[0:HID] = fw0[MT_OWN * 128 * NCORES:]

    in_maps = []
    for c in range(NCORES):
        own = W2[c * 1024:(c + 1) * 1024, :]                        # [1024, 16512]
        w2c = np.concatenate([own, shared_rows], axis=0)            # [1088, 16512]
        wcomb = np.concatenate(
            [np.ascontiguousarray(w2c.T), w1bt], axis=1)            # [16512, 1216]
        wq = np.ascontiguousarray(
            wcomb.reshape(QT, T3, 128, WC).transpose(0, 2, 1, 3)
            .reshape(QT * 128, T3 * WC)).astype(np.float16)
        fw0_t = np.zeros((128, NT), np.float32)
        for m in range(MT_OWN):
            fw0_t[:, m] = fw0[c * 1024 + m * 128: c * 1024 + (m + 1) * 128]
        fw0_t[:, NT - 1] = fw_shared
        cst = np.zeros((128, B + NT), np.float32)
        cst[:, 0:B] = xst
        cst[:, B:B + NT] = fw0_t
        sel = np.zeros((128, MT_OWN * HID + 1), np.float16)
        for m in range(MT_OWN):
            sel[:, m * HID + MT_OWN * c + m] = 1.0
        sel[:, MT_OWN * HID] = 1.0
        in_maps.append({"wq": wq, "cst": cst, "sel": sel})
    return in_maps


def kernel(x, W1, W2, fw0, _trace=False, _tmpdir=None):
    nc = _get_nc()
    _split_noops(nc)
    in_maps = _make_in_maps(x, W1, W2, fw0)
    res = run_bass_kernel_spmd(
        nc, in_maps, core_ids=list(range(NCORES)),
        trace=_trace, tmpdir=_tmpdir,
    )
    preds = np.zeros((1, B), np.float64)
    for c in range(NCORES):
        preds += res.results[c]["pred"].astype(np.float64)
    out = preds.astype(np.float32).reshape(B, 1)
    if _trace:
        return out, res
    return out
